# revision 42
# baseline (speedup 1.0000x reference)
"""Cross-attention Trainium2 kernel for nn_CrossAttention_37495064494692.

B=8 batches sharded 1/core across 8 NeuronCores (data parallel).
Per core: full cross-attention for one batch element in
feature-on-partitions ("transposed") layouts. Matmuls in bf16 with fp32
PSUM accumulation; softmax math fp32.

Host-side the text projection is folded into the kv projection:
  Wk = Wt @ Wkv[:, :E],  bk = bt @ Wkv[:, :E] + bkv[:E]   (same for v)
so the device computes
  KT  = Wk^T @ guideT + bk          [E, L]
  V   = guideT^T @ Wv + bv          [L, E]  stored per head with a ones
                                    column (v_aug) so the PV matmul also
                                    emits the softmax denominator row
  QT  = Wq^T @ queryT + bq          [E, S]  per 512-col s-chunk
  sT  = KT_h^T(l-tile) @ QT_h       [L, S]  two heads -> one 2-bank PSUM
                                    pair, one [128,1024] EXP each
  OTr_h = [v_h | 1]^T @ aT          [65, S] row 64 = denominator
  OT_h = OTr_h * bcast(1/denom)     bcast via K=1 matmul (no DRAM trip)
  out  = OT^T @ Wo + bo             [S, E]

The main loop software-pipelines across s-chunks: QT(c+1) and
out-projection(c-1) matmul chains are interleaved ("pumped") between the
score/PV bursts of chunk c so the PE never idles while the ACT engine
works through the EXPs.
"""
import sys

sys.path.insert(0, "/opt/trn_rl_repo")

from collections import deque

import ml_dtypes
import numpy as np

import concourse.bacc as bacc
import concourse.bass as bass
import concourse.tile as tile
from concourse import mybir
from concourse.bass_utils import run_bass_kernel_spmd

F32 = mybir.dt.float32
BF16 = mybir.dt.bfloat16
MMDT = BF16
NPDT = ml_dtypes.bfloat16

B, S, L = 8, 2048, 512
E, TE, H = 1024, 768, 16
D = E // H
SCALE = D ** -0.5

SC = 512              # s-chunk width
N_SC = S // SC        # 4 s-chunks
N_E = E // 128        # 8 E-chunks
N_TE = TE // 128      # 6 TE-chunks
N_LT = L // 128       # 4 L-tiles
HP = H // 2           # 8 head pairs

TRACE = False
_CACHED_NC = None


def build_nc():
    nc = bacc.Bacc()

    queryT = nc.declare_dram_parameter("queryT", [E, S], MMDT, isOutput=False)
    guideT = nc.declare_dram_parameter("guideT", [TE, L], MMDT, isOutput=False)
    Wq = nc.declare_dram_parameter("Wq", [E, E], MMDT, isOutput=False)
    Wk = nc.declare_dram_parameter("Wk", [TE, E], MMDT, isOutput=False)
    Wv = nc.declare_dram_parameter("Wv", [TE, E], MMDT, isOutput=False)
    Wo = nc.declare_dram_parameter("Wo", [E, E], MMDT, isOutput=False)
    bq = nc.declare_dram_parameter("bq", [E], F32, isOutput=False)
    bk = nc.declare_dram_parameter("bk", [E], F32, isOutput=False)
    bv_r = nc.declare_dram_parameter("bv_r", [E], MMDT, isOutput=False)
    bo = nc.declare_dram_parameter("bo", [E], F32, isOutput=False)
    mbias = nc.declare_dram_parameter("mbias", [L], F32, isOutput=False)
    selm = nc.declare_dram_parameter("selm", [40, H * 128 // 2], MMDT,
                                     isOutput=False)
    out = nc.declare_dram_parameter("out", [S, E], F32, isOutput=True)

    Exp = mybir.ActivationFunctionType.Exp

    with tile.TileContext(nc) as tc:
        with (
            tc.tile_pool(name="res", bufs=1) as res,
            tc.tile_pool(name="io", bufs=2) as io,
            tc.tile_pool(name="stp", bufs=3) as stp,
            tc.tile_pool(name="psA", bufs=2, space="PSUM") as psA,
            tc.tile_pool(name="psS", bufs=2, space="PSUM") as psS,
            tc.tile_pool(name="psV", bufs=2, space="PSUM") as psV,
        ):
            # ---- resident small tensors ----
            bq_sb = res.tile([128, N_E], F32, tag="bq")
            bk_sb = res.tile([128, N_E], F32, tag="bk")
            mb_sb = res.tile([128, N_LT], F32, tag="mb")
            nc.sync.dma_start(out=bq_sb, in_=bq.rearrange("(t p) -> p t", p=128))
            nc.sync.dma_start(out=bk_sb, in_=bk.rearrange("(t p) -> p t", p=128))
            nc.sync.dma_start(out=mb_sb, in_=mbias.rearrange("(t p) -> p t", p=128))
            bv_row = res.tile([1, E], MMDT, tag="bvr")
            nc.sync.dma_start(out=bv_row, in_=bv_r.rearrange("(one f) -> one f", one=1))
            ones_f = res.tile([1, 128], F32, tag="ones_f")
            ones_r = res.tile([1, 128], MMDT, tag="ones_r")
            nc.vector.memset(ones_f, 1.0)
            nc.scalar.copy(ones_r, ones_f)
            onesc_f = res.tile([128, H], F32, tag="onesc")
            nc.vector.memset(onesc_f, 1.0)
            # prewarm the ACT exp table set so the first real EXP is fast
            warm = res.tile([1, 8], F32, tag="warm")
            nc.scalar.activation(warm, ones_f[:, 0:8],
                                 mybir.ActivationFunctionType.Exp)
            # per-head-pair broadcast selector (heads 0-7 on rows 0-7, heads
            # 8-15 on rows 32-39 so both reciprocal halves start on legal
            # partitions; rows 8-31 are zero): bcast matmul SEL_hp^T @ recips
            # replicates head h's 1/denom row onto its 64 OT partitions
            SEL = res.tile([40, H * 128 // 2], MMDT, tag="SEL")
            nc.sync.dma_start(out=SEL, in_=selm[:, :])
            # denominator staging: head h parks at partition 32*(h//4),
            # col slot (h%4)*SC (DVE writes need 32-aligned start partitions)
            dn_st = res.tile([97, 4 * SC], MMDT, tag="dnst")

            # ---- long-lived activations + resident weights ----
            KT = [res.tile([128, L], MMDT, tag=f"KT{j}", name=f"KT{j}")
                  for j in range(N_E)]
            # v_aug: head h cols h*65..h*65+64, col h*65+64 == 1.0
            Vt = [res.tile([128, H * (D + 1)], MMDT, tag=f"V{lt}", name=f"V{lt}")
                  for lt in range(N_LT)]
            Wq_sb = [res.tile([128, E], MMDT, tag=f"wq{e}", name=f"wq{e}")
                     for e in range(N_E)]
            Wo_sb = [res.tile([128, E], MMDT, tag=f"wo{e}", name=f"wo{e}")
                     for e in range(N_E)]

            # chunk-0 query + weights prefetch, spread over four engine DMA
            # queues so no single queue gates the prologue
            qin0 = [io.tile([128, SC], MMDT, tag=f"qin{e}", name=f"qin{e}_0")
                    for e in range(N_E)]
            for e in range(N_E):
                nc.gpsimd.dma_start(out=qin0[e], in_=queryT[e * 128:(e + 1) * 128, 0:SC])
            for e in range(4, N_E):
                nc.gpsimd.dma_start(out=Wq_sb[e], in_=Wq[e * 128:(e + 1) * 128, :])
            for e in range(N_E):
                nc.gpsimd.dma_start(out=Wo_sb[e], in_=Wo[e * 128:(e + 1) * 128, :])
            bo_bc = res.tile([128, E], F32, tag="bo")
            bo_ap = bo[:]
            nc.gpsimd.dma_start(
                out=bo_bc,
                in_=bass.AP(tensor=bo_ap.tensor, offset=bo_ap.offset,
                            ap=[[0, 128], [1, E]]),
            )

            # ================= main loop =================
            fillers = deque()

            def pump(n):
                while n > 0 and fillers:
                    try:
                        next(fillers[0])
                        n -= 1
                    except StopIteration:
                        fillers.popleft()

            def drain():
                while fillers:
                    try:
                        next(fillers[0])
                    except StopIteration:
                        fillers.popleft()

            def emit_qt(c, qin, QTt, e_order=None):
                # QT = Wq^T @ queryT + bq, one yield per completed j-chain
                order = list(e_order) if e_order else list(range(N_E))
                for j in range(N_E):
                    ps = psA.tile([128, SC], F32, tag="acc", name=f"qacc{c}_{j}")
                    for i, e in enumerate(order):
                        nc.tensor.matmul(
                            ps, lhsT=Wq_sb[e][:, j * 128:(j + 1) * 128], rhs=qin[e],
                            start=(i == 0), stop=(i == N_E - 1),
                        )
                    nc.vector.tensor_scalar_add(QTt[j], ps, bq_sb[:, j:j + 1])
                    yield

            def emit_outproj(c, OTt):
                # out = OT^T @ Wo + bo, one yield per completed chain
                for st in range(N_LT):
                    sts = slice(st * 128, (st + 1) * 128)
                    for half in range(2):
                        ps = psA.tile([128, SC], F32, tag="acc",
                                      name=f"oacc{c}_{st}_{half}")
                        for j in range(N_E):
                            nc.tensor.matmul(
                                ps, lhsT=OTt[j][:, sts],
                                rhs=Wo_sb[j][:, half * SC:(half + 1) * SC],
                                start=(j == 0), stop=(j == N_E - 1),
                            )
                        ob = stp.tile([128, SC], F32, tag="ob",
                                      name=f"ob{c}_{st}_{half}")
                        nc.vector.tensor_add(
                            ob, ps, bo_bc[:, half * SC:(half + 1) * SC])
                        nc.sync.dma_start(
                            out=out[c * SC + st * 128:c * SC + (st + 1) * 128,
                                    half * SC:(half + 1) * SC],
                            in_=ob,
                        )
                        yield

            def emit_norm(c, OTrw, OTt, rca, hps):
                # per head pair: replicate 1/denom onto the 64 partitions of
                # each head (K=40 selector matmul), then scale the raw PV
                # output: OT = OTraw * bcast   (SBUF bf16 * PSUM f32)
                for hp in hps:
                    psB = psA.tile([128, SC], F32, tag="acc",
                                   name=f"bcp{c}_{hp}")
                    nc.tensor.matmul(
                        psB, lhsT=SEL[:, hp * 128:(hp + 1) * 128], rhs=rca,
                        start=True, stop=True,
                    )
                    nc.vector.tensor_mul(OTt[hp], OTrw[hp], psB)
                    yield

            def emit_scores(c, hp, QTc, atts, pump_lts=(1, 2, 3)):
                att = [io.tile([128, 2 * SC], MMDT, tag=f"att{lt}",
                               name=f"att{lt}_{c}_{hp}", bufs=4)
                       for lt in range(N_LT)]
                for lt in range(N_LT):
                    lts = slice(lt * 128, (lt + 1) * 128)
                    scp = psS.tile([128, 2 * SC], F32, tag="sc",
                                   name=f"sc{c}_{hp}_{lt}")
                    for u in range(2):
                        rows = slice(u * 64, (u + 1) * 64)
                        nc.tensor.matmul(
                            scp[:, u * SC:(u + 1) * SC],
                            lhsT=KT[hp][rows, lts], rhs=QTc[hp][rows, :],
                            start=True, stop=True,
                        )
                    nc.scalar.activation(
                        att[lt], scp, Exp,
                        bias=mb_sb[:, lt:lt + 1], scale=SCALE,
                    )
                    if lt in pump_lts:
                        pump(1)
                atts[hp] = att

            def emit_pv(c, hp, atts, OTraw):
                att = atts.pop(hp)
                for u in range(2):
                    h = 2 * hp + u
                    pv = psV.tile([D + 1, SC], F32, tag="pv", name=f"pv{h}_{c}")
                    for lt in range(N_LT):
                        nc.tensor.matmul(
                            pv, lhsT=Vt[lt][:, h * (D + 1):(h + 1) * (D + 1)],
                            rhs=att[lt][:, u * SC:(u + 1) * SC],
                            start=(lt == 0), stop=(lt == N_LT - 1),
                        )
                    nc.scalar.copy(
                        OTraw[hp][u * 64:(u + 1) * 64, :], pv[0:D, :])
                    nc.vector.tensor_copy(
                        dn_st[32 * (h // 4):32 * (h // 4) + 1,
                              (h % 4) * SC:(h % 4 + 1) * SC],
                        pv[D:D + 1, :])

            # ========== prologue: KT, V, QT(0), early chunk-0 scores ======
            QT_t = {0: [io.tile([128, SC], MMDT, tag=f"QT{j}", name=f"QT{j}_0")
                        for j in range(N_E)]}
            atts0 = {}
            with tc.tile_pool(name="pro", bufs=1) as pro:
                g_in = [pro.tile([128, L], MMDT, tag=f"gin{t}", name=f"gin{t}")
                        for t in range(N_TE)]
                Wk_sb = [pro.tile([128, E], MMDT, tag=f"wk{t}", name=f"wk{t}")
                         for t in range(N_TE)]
                Wv_sb = [pro.tile([128, E], MMDT, tag=f"wv{t}", name=f"wv{t}")
                         for t in range(N_TE)]
                for t in range(N_TE):
                    nc.sync.dma_start(out=g_in[t], in_=guideT[t * 128:(t + 1) * 128, :])
                for t in range(N_TE):
                    nc.scalar.dma_start(out=Wk_sb[t], in_=Wk[t * 128:(t + 1) * 128, :])
                for e in range(4):
                    nc.sync.dma_start(out=Wq_sb[e], in_=Wq[e * 128:(e + 1) * 128, :])
                for t in range(3):
                    nc.sync.dma_start(out=Wv_sb[t], in_=Wv[t * 128:(t + 1) * 128, :])
                for t in range(3, N_TE):
                    nc.scalar.dma_start(out=Wv_sb[t], in_=Wv[t * 128:(t + 1) * 128, :])

                def kt_chain(j):
                    # KT = Wk^T @ guideT + bk
                    ps = psA.tile([128, SC], F32, tag="acc", name=f"kacc{j}")
                    for t in range(N_TE):
                        nc.tensor.matmul(
                            ps, lhsT=Wk_sb[t][:, j * 128:(j + 1) * 128], rhs=g_in[t],
                            start=(t == 0), stop=(t == N_TE - 1),
                        )
                    nc.vector.tensor_scalar_add(KT[j], ps, bk_sb[:, j:j + 1])

                # KT chains first (their DMAs land earliest), then interleave
                # QT(0) chains and the first three chunk-0 score/EXP blocks
                qt0_gen = emit_qt(0, qin0, QT_t[0])
                kt_chain(0)
                kt_chain(1)
                kt_chain(2)
                kt_chain(3)
                next(qt0_gen)
                emit_scores(0, 0, QT_t[0], atts0)
                kt_chain(4); next(qt0_gen)
                emit_scores(0, 1, QT_t[0], atts0)
                kt_chain(5); next(qt0_gen)
                emit_scores(0, 2, QT_t[0], atts0)
                kt_chain(6); next(qt0_gen)
                kt_chain(7); next(qt0_gen)
                for _ in qt0_gen:
                    pass

                # V = guideT^T @ Wv + bv, into v_aug layout
                for lt in range(N_LT):
                    va = Vt[lt].rearrange("p (h c) -> p h c", c=D + 1)
                    for half in range(2):
                        ps = psA.tile([128, SC], F32, tag="acc", name=f"vacc{lt}_{half}")
                        for t in range(N_TE):
                            nc.tensor.matmul(
                                ps, lhsT=g_in[t][:, lt * 128:(lt + 1) * 128],
                                rhs=Wv_sb[t][:, half * SC:(half + 1) * SC],
                                start=(t == 0), stop=False,
                            )
                        nc.tensor.matmul(
                            ps, lhsT=ones_r,
                            rhs=bv_row[:, half * SC:(half + 1) * SC],
                            start=False, stop=True,
                        )
                        nc.vector.tensor_copy(
                            va[:, half * 8:(half + 1) * 8, 0:D],
                            ps.rearrange("p (h c) -> p h c", c=D),
                        )
                    nc.vector.tensor_copy(
                        va[:, :, D:D + 1],
                        onesc_f.rearrange("p (h c) -> p h c", c=1),
                    )

            def make_finalize(c, OTraw, OTc):
                # Finalization is split: heads 0-7 right after PV(c,3) and
                # heads 8-15 after PV(c,7), each a gather DMA + batched
                # reciprocal, so the selector matmuls never wait on a recip.
                state = {}

                def finA():
                    rca = stp.tile([40, SC], MMDT, tag="rca", name=f"rca{c}")
                    dn_c = stp.tile([40, SC], MMDT, tag="dna", name=f"dn_{c}")
                    nc.vector.memset(rca, 0.0)
                    nc.sync.dma_start(out=dn_c[0:4, :], in_=dn_st[0:1, :])
                    nc.sync.dma_start(out=dn_c[4:8, :], in_=dn_st[32:33, :])
                    with nc.allow_low_precision(reason="bf16 recip of denom"):
                        nc.vector.reciprocal(rca[0:8, :], dn_c[0:8, :])
                    state["rca"], state["dn"] = rca, dn_c
                    fillers.append(emit_norm(c, OTraw, OTc, rca, range(0, 4)))

                def finB():
                    rca, dn_c = state["rca"], state["dn"]
                    nc.sync.dma_start(out=dn_c[32:36, :], in_=dn_st[64:65, :])
                    nc.sync.dma_start(out=dn_c[36:40, :], in_=dn_st[96:97, :])
                    with nc.allow_low_precision(reason="bf16 recip of denom"):
                        nc.vector.reciprocal(rca[32:40, :], dn_c[32:40, :])
                    fillers.append(emit_norm(c, OTraw, OTc, rca, range(4, HP)))
                    fillers.append(emit_outproj(c, OTc))
                return finA, finB

            # The last two PV chains of each chunk (and that chunk's
            # finalize) carry over into the next chunk's first score slots so
            # the score/EXP stream never pauses at a chunk boundary.
            carry = []
            for c in range(N_SC):
                if c + 1 < N_SC:
                    qin = [io.tile([128, SC], MMDT, tag=f"qin{e}",
                                   name=f"qin{e}_{c + 1}") for e in range(N_E)]
                    for e in range(N_E):
                        (nc.sync if e % 2 == 0 else nc.gpsimd).dma_start(
                            out=qin[e],
                            in_=queryT[e * 128:(e + 1) * 128,
                                       (c + 1) * SC:(c + 2) * SC])
                    QT_t[c + 1] = [io.tile([128, SC], MMDT, tag=f"QT{j}",
                                           name=f"QT{j}_{c + 1}")
                                   for j in range(N_E)]
                    fillers.append(emit_qt(c + 1, qin, QT_t[c + 1]))
                OTraw = [io.tile([128, SC], MMDT, tag=f"OTr{j}",
                                 name=f"OTr{j}_{c}") for j in range(N_E)]
                OTc = [io.tile([128, SC], MMDT, tag=f"OT{j}", name=f"OT{j}_{c}")
                       for j in range(N_E)]
                QTc = QT_t.pop(c)

                atts = atts0 if c == 0 else {}
                n_own = HP if c == N_SC - 1 else HP - 2
                finA, finB = make_finalize(c, OTraw, OTc)
                fin_map = {3: finA, HP - 1: finB}
                pv_queue = carry + [(c, hp, atts, OTraw, fin_map.get(hp))
                                    for hp in range(n_own)]
                carry = [(c, hp, atts, OTraw, fin_map.get(hp))
                         for hp in range(n_own, HP)]

                # pump pacing: chunk 0 has only QT(1) as filler, so spread it
                # one chain per head pair; chunks 1-2 absorb the spillover
                # with an extra pump slot; the last chunk stays starved so
                # leftover chains cover the final reciprocal at drain time
                if c == 0 or c == N_SC - 1:
                    plts = (3,)
                else:
                    plts = (0, 1, 2, 3)
                for hp in range(3 if c == 0 else 0, HP):
                    emit_scores(c, hp, QTc, atts, pump_lts=plts)
                    if pv_queue:
                        pc, php, patts, pOTraw, pfin = pv_queue.pop(0)
                        emit_pv(pc, php, patts, pOTraw)
                        if pfin is not None:
                            pfin()
                while pv_queue:
                    pc, php, patts, pOTraw, pfin = pv_queue.pop(0)
                    emit_pv(pc, php, patts, pOTraw)
                    if pfin is not None:
                        pfin()
            drain()

    if not nc.is_finalized():
        nc.finalize()
    return nc


def kernel(query, guide_vector, attention_mask, Wt, bt, Wq, bq, Wkv, bkv, Wo, bo):
    global _CACHED_NC
    query = np.asarray(query, dtype=np.float32)
    guide_vector = np.asarray(guide_vector, dtype=np.float32)
    attention_mask = np.asarray(attention_mask)
    Wt = np.asarray(Wt, dtype=np.float32)
    bt = np.asarray(bt, dtype=np.float32)
    bq = np.asarray(bq, dtype=np.float32)
    bkv = np.asarray(bkv, dtype=np.float32)
    bo = np.asarray(bo, dtype=np.float32)
    Wkv = np.asarray(Wkv, dtype=np.float32)

    # fold the text projection into the kv projection (host-side, fp32)
    Wf = Wt @ Wkv                       # [TE, 2E]
    bf = bt @ Wkv + bkv                 # [2E]
    Wk_m = np.ascontiguousarray(Wf[:, :E]).astype(NPDT)
    Wv_m = np.ascontiguousarray(Wf[:, E:]).astype(NPDT)
    Wq_m = np.asarray(Wq, dtype=np.float32).astype(NPDT)
    Wo_m = np.asarray(Wo, dtype=np.float32).astype(NPDT)
    bk_m = np.ascontiguousarray(bf[:E])
    bv_m = bf[E:].astype(NPDT)

    if _CACHED_NC is None:
        _CACHED_NC = build_nc()
    nc = _CACHED_NC

    selm = np.zeros((40, H * 128 // 2), dtype=NPDT)
    for h in range(H):
        row = h if h < 8 else 24 + h
        col = (h // 2) * 128 + (h % 2) * 64
        selm[row, col:col + 64] = 1.0

    mb = np.where(attention_mask == 0, np.float32(-1e9), np.float32(0.0))
    in_maps = []
    for b in range(B):
        in_maps.append({
            "queryT": np.ascontiguousarray(query[b].T).astype(NPDT),
            "guideT": np.ascontiguousarray(guide_vector[b].T).astype(NPDT),
            "Wq": Wq_m, "Wk": Wk_m, "Wv": Wv_m, "Wo": Wo_m,
            "bq": bq, "bk": bk_m, "bv_r": bv_m,
            "bo": bo, "mbias": mb[b].astype(np.float32), "selm": selm,
        })
    res = run_bass_kernel_spmd(nc, in_maps, list(range(B)), trace=TRACE)
    if TRACE:
        kernel.last_exec_time_ns = res.exec_time_ns
        kernel.last_results = res
    return np.stack([res.results[b]["out"] for b in range(B)])


# revision 44
# speedup vs baseline: 1.0507x; 1.0507x over previous
"""Cross-attention Trainium2 kernel for nn_CrossAttention_37495064494692.

B=8 batches sharded 1/core across 8 NeuronCores (data parallel).
Per core: full cross-attention for one batch element in
feature-on-partitions ("transposed") layouts. Matmuls in bf16 with fp32
PSUM accumulation; softmax math fp32.

Host-side the text projection is folded into the kv projection:
  Wk = Wt @ Wkv[:, :E],  bk = bt @ Wkv[:, :E] + bkv[:E]   (same for v)
so the device computes
  KT  = Wk^T @ guideT + bk          [E, L]
  V   = guideT^T @ Wv + bv          [L, E]  stored per head with a ones
                                    column (v_aug) so the PV matmul also
                                    emits the softmax denominator row
  QT  = Wq^T @ queryT + bq          [E, S]  per 512-col s-chunk
  sT  = KT_h^T(l-tile) @ QT_h       [L, S]  two heads -> one 2-bank PSUM
                                    pair, one [128,1024] EXP each
  OTr_h = [v_h | 1]^T @ aT          [65, S] row 64 = denominator
  OT_h = OTr_h * bcast(1/denom)     bcast via K=1 matmul (no DRAM trip)
  out  = OT^T @ Wo + bo             [S, E]

The main loop software-pipelines across s-chunks: QT(c+1) and
out-projection(c-1) matmul chains are interleaved ("pumped") between the
score/PV bursts of chunk c so the PE never idles while the ACT engine
works through the EXPs.
"""
import sys

sys.path.insert(0, "/opt/trn_rl_repo")

from collections import deque

import ml_dtypes
import numpy as np

import concourse.bacc as bacc
import concourse.bass as bass
import concourse.tile as tile
from concourse import mybir
from concourse.bass_utils import run_bass_kernel_spmd

F32 = mybir.dt.float32
BF16 = mybir.dt.bfloat16
MMDT = BF16
NPDT = ml_dtypes.bfloat16

B, S, L = 8, 2048, 512
E, TE, H = 1024, 768, 16
D = E // H
SCALE = D ** -0.5

SC = 512              # s-chunk width
N_SC = S // SC        # 4 s-chunks
N_E = E // 128        # 8 E-chunks
N_TE = TE // 128      # 6 TE-chunks
N_LT = L // 128       # 4 L-tiles
HP = H // 2           # 8 head pairs

TRACE = False
_CACHED_NC = None


def build_nc():
    nc = bacc.Bacc()

    queryT = nc.declare_dram_parameter("queryT", [E, S], MMDT, isOutput=False)
    guideT = nc.declare_dram_parameter("guideT", [TE, L], MMDT, isOutput=False)
    Wq = nc.declare_dram_parameter("Wq", [E, E], MMDT, isOutput=False)
    Wk = nc.declare_dram_parameter("Wk", [TE, E], MMDT, isOutput=False)
    Wv = nc.declare_dram_parameter("Wv", [TE, E], MMDT, isOutput=False)
    Wo = nc.declare_dram_parameter("Wo", [E, E], MMDT, isOutput=False)
    bq = nc.declare_dram_parameter("bq", [E], F32, isOutput=False)
    bk = nc.declare_dram_parameter("bk", [E], F32, isOutput=False)
    bv_r = nc.declare_dram_parameter("bv_r", [E], MMDT, isOutput=False)
    bo = nc.declare_dram_parameter("bo", [E], F32, isOutput=False)
    mbias = nc.declare_dram_parameter("mbias", [L], F32, isOutput=False)
    selm = nc.declare_dram_parameter("selm", [40, H * 128 // 2], MMDT,
                                     isOutput=False)
    out = nc.declare_dram_parameter("out", [S, E], F32, isOutput=True)

    Exp = mybir.ActivationFunctionType.Exp

    with tile.TileContext(nc) as tc:
        with (
            tc.tile_pool(name="res", bufs=1) as res,
            tc.tile_pool(name="io", bufs=2) as io,
            tc.tile_pool(name="stp", bufs=3) as stp,
            tc.tile_pool(name="psA", bufs=2, space="PSUM") as psA,
            tc.tile_pool(name="psS", bufs=2, space="PSUM") as psS,
            tc.tile_pool(name="psV", bufs=2, space="PSUM") as psV,
        ):
            # ---- resident small tensors ----
            bq_sb = res.tile([128, N_E], F32, tag="bq")
            bk_sb = res.tile([128, N_E], F32, tag="bk")
            mb_sb = res.tile([128, N_LT], F32, tag="mb")
            nc.sync.dma_start(out=bq_sb, in_=bq.rearrange("(t p) -> p t", p=128))
            nc.sync.dma_start(out=bk_sb, in_=bk.rearrange("(t p) -> p t", p=128))
            nc.sync.dma_start(out=mb_sb, in_=mbias.rearrange("(t p) -> p t", p=128))
            bv_row = res.tile([1, E], MMDT, tag="bvr")
            nc.sync.dma_start(out=bv_row, in_=bv_r.rearrange("(one f) -> one f", one=1))
            ones_f = res.tile([1, 128], F32, tag="ones_f")
            ones_r = res.tile([1, 128], MMDT, tag="ones_r")
            nc.vector.memset(ones_f, 1.0)
            nc.scalar.copy(ones_r, ones_f)
            onesc_f = res.tile([128, H], F32, tag="onesc")
            nc.vector.memset(onesc_f, 1.0)
            # prewarm the ACT exp table set so the first real EXP is fast
            warm = res.tile([1, 8], F32, tag="warm")
            nc.scalar.activation(warm, ones_f[:, 0:8],
                                 mybir.ActivationFunctionType.Exp)
            # per-head-pair broadcast selector (heads 0-7 on rows 0-7, heads
            # 8-15 on rows 32-39 so both reciprocal halves start on legal
            # partitions; rows 8-31 are zero): bcast matmul SEL_hp^T @ recips
            # replicates head h's 1/denom row onto its 64 OT partitions
            SEL = res.tile([40, H * 128 // 2], MMDT, tag="SEL")
            nc.sync.dma_start(out=SEL, in_=selm[:, :])
            # denominator staging: head h parks at partition 32*(h//4),
            # col slot (h%4)*SC (DVE writes need 32-aligned start partitions)
            dn_st = res.tile([97, 4 * SC], MMDT, tag="dnst")

            # ---- long-lived activations + resident weights ----
            KT = [res.tile([128, L], MMDT, tag=f"KT{j}", name=f"KT{j}")
                  for j in range(N_E)]
            # v_aug: head h cols h*65..h*65+64, col h*65+64 == 1.0
            Vt = [res.tile([128, H * (D + 1)], MMDT, tag=f"V{lt}", name=f"V{lt}")
                  for lt in range(N_LT)]
            Wq_sb = [res.tile([128, E], MMDT, tag=f"wq{e}", name=f"wq{e}")
                     for e in range(N_E)]
            Wo_sb = [res.tile([128, E], MMDT, tag=f"wo{e}", name=f"wo{e}")
                     for e in range(N_E)]

            # chunk-0 query + weights prefetch, spread over four engine DMA
            # queues so no single queue gates the prologue
            qin0 = [io.tile([128, SC], MMDT, tag=f"qin{e}", name=f"qin{e}_0")
                    for e in range(N_E)]
            for e in range(N_E):
                nc.gpsimd.dma_start(out=qin0[e], in_=queryT[e * 128:(e + 1) * 128, 0:SC])
            for e in range(4, N_E):
                nc.gpsimd.dma_start(out=Wq_sb[e], in_=Wq[e * 128:(e + 1) * 128, :])
            for e in range(N_E):
                nc.gpsimd.dma_start(out=Wo_sb[e], in_=Wo[e * 128:(e + 1) * 128, :])
            bo_bc = res.tile([128, E], F32, tag="bo")
            bo_ap = bo[:]
            nc.gpsimd.dma_start(
                out=bo_bc,
                in_=bass.AP(tensor=bo_ap.tensor, offset=bo_ap.offset,
                            ap=[[0, 128], [1, E]]),
            )

            # ================= main loop =================
            fillers = deque()

            def pump(n):
                while n > 0 and fillers:
                    try:
                        next(fillers[0])
                        n -= 1
                    except StopIteration:
                        fillers.popleft()

            def drain():
                while fillers:
                    try:
                        next(fillers[0])
                    except StopIteration:
                        fillers.popleft()

            def emit_qt(c, qin, QTt, e_order=None):
                # QT = Wq^T @ queryT + bq, one yield per completed j-chain
                order = list(e_order) if e_order else list(range(N_E))
                for j in range(N_E):
                    ps = psA.tile([128, SC], F32, tag="acc", name=f"qacc{c}_{j}")
                    for i, e in enumerate(order):
                        nc.tensor.matmul(
                            ps, lhsT=Wq_sb[e][:, j * 128:(j + 1) * 128], rhs=qin[e],
                            start=(i == 0), stop=(i == N_E - 1),
                        )
                    nc.vector.tensor_scalar_add(QTt[j], ps, bq_sb[:, j:j + 1])
                    yield

            def emit_outproj(c, OTt):
                # out = OT^T @ Wo + bo, one yield per completed chain
                for st in range(N_LT):
                    sts = slice(st * 128, (st + 1) * 128)
                    for half in range(2):
                        ps = psA.tile([128, SC], F32, tag="acc",
                                      name=f"oacc{c}_{st}_{half}")
                        for j in range(N_E):
                            nc.tensor.matmul(
                                ps, lhsT=OTt[j][:, sts],
                                rhs=Wo_sb[j][:, half * SC:(half + 1) * SC],
                                start=(j == 0), stop=(j == N_E - 1),
                            )
                        ob = stp.tile([128, SC], F32, tag="ob",
                                      name=f"ob{c}_{st}_{half}")
                        nc.vector.tensor_add(
                            ob, ps, bo_bc[:, half * SC:(half + 1) * SC])
                        nc.sync.dma_start(
                            out=out[c * SC + st * 128:c * SC + (st + 1) * 128,
                                    half * SC:(half + 1) * SC],
                            in_=ob,
                        )
                        yield

            def emit_norm(c, OTrw, OTt, rca, hps):
                # per head pair: replicate 1/denom onto the 64 partitions of
                # each head (K=40 selector matmul), then scale the raw PV
                # output: OT = OTraw * bcast   (SBUF bf16 * PSUM f32)
                for hp in hps:
                    psB = psA.tile([128, SC], F32, tag="acc",
                                   name=f"bcp{c}_{hp}")
                    nc.tensor.matmul(
                        psB, lhsT=SEL[:, hp * 128:(hp + 1) * 128], rhs=rca,
                        start=True, stop=True,
                    )
                    nc.vector.tensor_mul(OTt[hp], OTrw[hp], psB)
                    yield

            def emit_scores(c, hp, QTc, atts, pump_lts=(1, 2, 3)):
                att = [io.tile([128, 2 * SC], MMDT, tag=f"att{lt}",
                               name=f"att{lt}_{c}_{hp}", bufs=4)
                       for lt in range(N_LT)]
                for lt in range(N_LT):
                    lts = slice(lt * 128, (lt + 1) * 128)
                    scp = psS.tile([128, 2 * SC], F32, tag="sc",
                                   name=f"sc{c}_{hp}_{lt}")
                    for u in range(2):
                        rows = slice(u * 64, (u + 1) * 64)
                        nc.tensor.matmul(
                            scp[:, u * SC:(u + 1) * SC],
                            lhsT=KT[hp][rows, lts], rhs=QTc[hp][rows, :],
                            start=True, stop=True,
                        )
                    nc.scalar.activation(
                        att[lt], scp, Exp,
                        bias=mb_sb[:, lt:lt + 1], scale=SCALE,
                    )
                    if lt in pump_lts:
                        pump(1)
                atts[hp] = att

            def emit_pv(c, hp, atts, OTraw):
                att = atts.pop(hp)
                for u in range(2):
                    h = 2 * hp + u
                    pv = psV.tile([D + 1, SC], F32, tag="pv", name=f"pv{h}_{c}")
                    for lt in range(N_LT):
                        nc.tensor.matmul(
                            pv, lhsT=Vt[lt][:, h * (D + 1):(h + 1) * (D + 1)],
                            rhs=att[lt][:, u * SC:(u + 1) * SC],
                            start=(lt == 0), stop=(lt == N_LT - 1),
                        )
                    nc.scalar.copy(
                        OTraw[hp][u * 64:(u + 1) * 64, :], pv[0:D, :])
                    nc.vector.tensor_copy(
                        dn_st[32 * (h // 4):32 * (h // 4) + 1,
                              (h % 4) * SC:(h % 4 + 1) * SC],
                        pv[D:D + 1, :])

            # ========== prologue: KT, V, QT(0), early chunk-0 scores ======
            QT_t = {0: [io.tile([128, SC], MMDT, tag=f"QT{j}", name=f"QT{j}_0")
                        for j in range(N_E)]}
            atts0 = {}
            with tc.tile_pool(name="pro", bufs=1) as pro:
                g_in = [pro.tile([128, L], MMDT, tag=f"gin{t}", name=f"gin{t}")
                        for t in range(N_TE)]
                Wk_sb = [pro.tile([128, E], MMDT, tag=f"wk{t}", name=f"wk{t}")
                         for t in range(N_TE)]
                Wv_sb = [pro.tile([128, E], MMDT, tag=f"wv{t}", name=f"wv{t}")
                         for t in range(N_TE)]
                for t in range(N_TE):
                    nc.sync.dma_start(out=g_in[t], in_=guideT[t * 128:(t + 1) * 128, :])
                for t in range(N_TE):
                    nc.scalar.dma_start(out=Wk_sb[t], in_=Wk[t * 128:(t + 1) * 128, :])
                for e in range(4):
                    nc.sync.dma_start(out=Wq_sb[e], in_=Wq[e * 128:(e + 1) * 128, :])
                for t in range(3):
                    nc.sync.dma_start(out=Wv_sb[t], in_=Wv[t * 128:(t + 1) * 128, :])
                for t in range(3, N_TE):
                    nc.scalar.dma_start(out=Wv_sb[t], in_=Wv[t * 128:(t + 1) * 128, :])

                def kt_chain(j):
                    # KT = Wk^T @ guideT + bk
                    ps = psA.tile([128, SC], F32, tag="acc", name=f"kacc{j}")
                    for t in range(N_TE):
                        nc.tensor.matmul(
                            ps, lhsT=Wk_sb[t][:, j * 128:(j + 1) * 128], rhs=g_in[t],
                            start=(t == 0), stop=(t == N_TE - 1),
                        )
                    nc.vector.tensor_scalar_add(KT[j], ps, bk_sb[:, j:j + 1])

                # KT chains first (their DMAs land earliest), then interleave
                # QT(0) chains and the first three chunk-0 score/EXP blocks
                qt0_gen = emit_qt(0, qin0, QT_t[0])
                kt_chain(0)
                kt_chain(1)
                kt_chain(2)
                kt_chain(3)
                next(qt0_gen)
                emit_scores(0, 0, QT_t[0], atts0)
                kt_chain(4); next(qt0_gen)
                emit_scores(0, 1, QT_t[0], atts0)
                kt_chain(5); next(qt0_gen)
                emit_scores(0, 2, QT_t[0], atts0)
                kt_chain(6); next(qt0_gen)
                kt_chain(7); next(qt0_gen)
                for _ in qt0_gen:
                    pass

                # V = guideT^T @ Wv + bv, into v_aug layout
                for lt in range(N_LT):
                    va = Vt[lt].rearrange("p (h c) -> p h c", c=D + 1)
                    for half in range(2):
                        ps = psA.tile([128, SC], F32, tag="acc", name=f"vacc{lt}_{half}")
                        for t in range(N_TE):
                            nc.tensor.matmul(
                                ps, lhsT=g_in[t][:, lt * 128:(lt + 1) * 128],
                                rhs=Wv_sb[t][:, half * SC:(half + 1) * SC],
                                start=(t == 0), stop=False,
                            )
                        nc.tensor.matmul(
                            ps, lhsT=ones_r,
                            rhs=bv_row[:, half * SC:(half + 1) * SC],
                            start=False, stop=True,
                        )
                        nc.vector.tensor_copy(
                            va[:, half * 8:(half + 1) * 8, 0:D],
                            ps.rearrange("p (h c) -> p h c", c=D),
                        )
                    nc.vector.tensor_copy(
                        va[:, :, D:D + 1],
                        onesc_f.rearrange("p (h c) -> p h c", c=1),
                    )

            def make_finalize(c, OTraw, OTc):
                # Finalization is split: heads 0-7 right after PV(c,3) and
                # heads 8-15 after PV(c,7), each a gather DMA + batched
                # reciprocal, so the selector matmuls never wait on a recip.
                state = {}

                def finA():
                    rca = stp.tile([40, SC], MMDT, tag="rca", name=f"rca{c}")
                    dn_c = stp.tile([40, SC], MMDT, tag="dna", name=f"dn_{c}")
                    nc.vector.memset(rca, 0.0)
                    nc.sync.dma_start(out=dn_c[0:4, :], in_=dn_st[0:1, :])
                    nc.sync.dma_start(out=dn_c[4:8, :], in_=dn_st[32:33, :])
                    with nc.allow_low_precision(reason="bf16 recip of denom"):
                        nc.vector.reciprocal(rca[0:8, :], dn_c[0:8, :])
                    state["rca"], state["dn"] = rca, dn_c
                    fillers.append(emit_norm(c, OTraw, OTc, rca, range(0, 4)))

                def finB():
                    rca, dn_c = state["rca"], state["dn"]
                    nc.sync.dma_start(out=dn_c[32:36, :], in_=dn_st[64:65, :])
                    nc.sync.dma_start(out=dn_c[36:40, :], in_=dn_st[96:97, :])
                    with nc.allow_low_precision(reason="bf16 recip of denom"):
                        nc.vector.reciprocal(rca[32:40, :], dn_c[32:40, :])
                    fillers.append(emit_norm(c, OTraw, OTc, rca, range(4, HP)))
                    fillers.append(emit_outproj(c, OTc))
                return finA, finB

            # The last two PV chains of each chunk (and that chunk's
            # finalize) carry over into the next chunk's first score slots so
            # the score/EXP stream never pauses at a chunk boundary.
            carry = []
            for c in range(N_SC):
                if c + 1 < N_SC:
                    qin = [io.tile([128, SC], MMDT, tag=f"qin{e}",
                                   name=f"qin{e}_{c + 1}") for e in range(N_E)]
                    for e in range(N_E):
                        (nc.sync if e % 2 == 0 else nc.gpsimd).dma_start(
                            out=qin[e],
                            in_=queryT[e * 128:(e + 1) * 128,
                                       (c + 1) * SC:(c + 2) * SC])
                    QT_t[c + 1] = [io.tile([128, SC], MMDT, tag=f"QT{j}",
                                           name=f"QT{j}_{c + 1}")
                                   for j in range(N_E)]
                    fillers.append(emit_qt(c + 1, qin, QT_t[c + 1]))
                OTraw = [io.tile([128, SC], MMDT, tag=f"OTr{j}",
                                 name=f"OTr{j}_{c}") for j in range(N_E)]
                OTc = [io.tile([128, SC], MMDT, tag=f"OT{j}", name=f"OT{j}_{c}")
                       for j in range(N_E)]
                QTc = QT_t.pop(c)

                atts = atts0 if c == 0 else {}
                n_own = HP if c == N_SC - 1 else HP - 2
                finA, finB = make_finalize(c, OTraw, OTc)
                fin_map = {3: finA, HP - 1: finB}
                pv_queue = carry + [(c, hp, atts, OTraw, fin_map.get(hp))
                                    for hp in range(n_own)]
                carry = [(c, hp, atts, OTraw, fin_map.get(hp))
                         for hp in range(n_own, HP)]

                for hp in range(3 if c == 0 else 0, HP):
                    if c == 0:
                        # chunk 0's only filler is the 8 QT(1) chains: spread
                        # them 2,2,2,1,1 over its five score slots so the
                        # back half does not starve and go ACT-bound
                        plts = (1, 3) if hp <= 5 else (3,)
                    elif c == N_SC - 1:
                        plts = (3,)
                    else:
                        plts = (1, 2, 3)
                    emit_scores(c, hp, QTc, atts, pump_lts=plts)
                    if pv_queue:
                        pc, php, patts, pOTraw, pfin = pv_queue.pop(0)
                        emit_pv(pc, php, patts, pOTraw)
                        if pfin is not None:
                            pfin()
                while pv_queue:
                    pc, php, patts, pOTraw, pfin = pv_queue.pop(0)
                    emit_pv(pc, php, patts, pOTraw)
                    if pfin is not None:
                        pfin()
            drain()

    if not nc.is_finalized():
        nc.finalize()
    return nc


def kernel(query, guide_vector, attention_mask, Wt, bt, Wq, bq, Wkv, bkv, Wo, bo):
    global _CACHED_NC
    query = np.asarray(query, dtype=np.float32)
    guide_vector = np.asarray(guide_vector, dtype=np.float32)
    attention_mask = np.asarray(attention_mask)
    Wt = np.asarray(Wt, dtype=np.float32)
    bt = np.asarray(bt, dtype=np.float32)
    bq = np.asarray(bq, dtype=np.float32)
    bkv = np.asarray(bkv, dtype=np.float32)
    bo = np.asarray(bo, dtype=np.float32)
    Wkv = np.asarray(Wkv, dtype=np.float32)

    # fold the text projection into the kv projection (host-side, fp32)
    Wf = Wt @ Wkv                       # [TE, 2E]
    bf = bt @ Wkv + bkv                 # [2E]
    Wk_m = np.ascontiguousarray(Wf[:, :E]).astype(NPDT)
    Wv_m = np.ascontiguousarray(Wf[:, E:]).astype(NPDT)
    Wq_m = np.asarray(Wq, dtype=np.float32).astype(NPDT)
    Wo_m = np.asarray(Wo, dtype=np.float32).astype(NPDT)
    bk_m = np.ascontiguousarray(bf[:E])
    bv_m = bf[E:].astype(NPDT)

    if _CACHED_NC is None:
        _CACHED_NC = build_nc()
    nc = _CACHED_NC

    selm = np.zeros((40, H * 128 // 2), dtype=NPDT)
    for h in range(H):
        row = h if h < 8 else 24 + h
        col = (h // 2) * 128 + (h % 2) * 64
        selm[row, col:col + 64] = 1.0

    mb = np.where(attention_mask == 0, np.float32(-1e9), np.float32(0.0))
    in_maps = []
    for b in range(B):
        in_maps.append({
            "queryT": np.ascontiguousarray(query[b].T).astype(NPDT),
            "guideT": np.ascontiguousarray(guide_vector[b].T).astype(NPDT),
            "Wq": Wq_m, "Wk": Wk_m, "Wv": Wv_m, "Wo": Wo_m,
            "bq": bq, "bk": bk_m, "bv_r": bv_m,
            "bo": bo, "mbias": mb[b].astype(np.float32), "selm": selm,
        })
    res = run_bass_kernel_spmd(nc, in_maps, list(range(B)), trace=TRACE)
    if TRACE:
        kernel.last_exec_time_ns = res.exec_time_ns
        kernel.last_results = res
    return np.stack([res.results[b]["out"] for b in range(B)])


# revision 47
# speedup vs baseline: 1.0535x; 1.0027x over previous
"""Cross-attention Trainium2 kernel for nn_CrossAttention_37495064494692.

B=8 batches sharded 1/core across 8 NeuronCores (data parallel).
Per core: full cross-attention for one batch element in
feature-on-partitions ("transposed") layouts. Matmuls in bf16 with fp32
PSUM accumulation; softmax math fp32.

Host-side the text projection is folded into the kv projection:
  Wk = Wt @ Wkv[:, :E],  bk = bt @ Wkv[:, :E] + bkv[:E]   (same for v)
so the device computes
  KT  = Wk^T @ guideT + bk          [E, L]
  V   = guideT^T @ Wv + bv          [L, E]  stored per head with a ones
                                    column (v_aug) so the PV matmul also
                                    emits the softmax denominator row
  QT  = Wq^T @ queryT + bq          [E, S]  per 512-col s-chunk
  sT  = KT_h^T(l-tile) @ QT_h       [L, S]  two heads -> one 2-bank PSUM
                                    pair, one [128,1024] EXP each
  OTr_h = [v_h | 1]^T @ aT          [65, S] row 64 = denominator
  OT_h = OTr_h * bcast(1/denom)     bcast via K=1 matmul (no DRAM trip)
  out  = OT^T @ Wo + bo             [S, E]

The main loop software-pipelines across s-chunks: QT(c+1) and
out-projection(c-1) matmul chains are interleaved ("pumped") between the
score/PV bursts of chunk c so the PE never idles while the ACT engine
works through the EXPs.
"""
import sys

sys.path.insert(0, "/opt/trn_rl_repo")

from collections import deque

import ml_dtypes
import numpy as np

import concourse.bacc as bacc
import concourse.bass as bass
import concourse.tile as tile
from concourse import mybir
from concourse.bass_utils import run_bass_kernel_spmd

F32 = mybir.dt.float32
BF16 = mybir.dt.bfloat16
MMDT = BF16
NPDT = ml_dtypes.bfloat16

B, S, L = 8, 2048, 512
E, TE, H = 1024, 768, 16
D = E // H
SCALE = D ** -0.5

SC = 512              # s-chunk width
N_SC = S // SC        # 4 s-chunks
N_E = E // 128        # 8 E-chunks
N_TE = TE // 128      # 6 TE-chunks
N_LT = L // 128       # 4 L-tiles
HP = H // 2           # 8 head pairs

TRACE = False
_CACHED_NC = None


def build_nc():
    nc = bacc.Bacc()

    queryT = nc.declare_dram_parameter("queryT", [E, S], MMDT, isOutput=False)
    guideT = nc.declare_dram_parameter("guideT", [TE, L], MMDT, isOutput=False)
    Wq = nc.declare_dram_parameter("Wq", [E, E], MMDT, isOutput=False)
    Wk = nc.declare_dram_parameter("Wk", [TE, E], MMDT, isOutput=False)
    Wv = nc.declare_dram_parameter("Wv", [TE, E], MMDT, isOutput=False)
    Wo = nc.declare_dram_parameter("Wo", [E, E], MMDT, isOutput=False)
    bq = nc.declare_dram_parameter("bq", [E], F32, isOutput=False)
    bk = nc.declare_dram_parameter("bk", [E], F32, isOutput=False)
    bv_r = nc.declare_dram_parameter("bv_r", [E], MMDT, isOutput=False)
    bo = nc.declare_dram_parameter("bo", [E], F32, isOutput=False)
    mbias = nc.declare_dram_parameter("mbias", [L], F32, isOutput=False)
    selm = nc.declare_dram_parameter("selm", [40, H * 128 // 2], MMDT,
                                     isOutput=False)
    out = nc.declare_dram_parameter("out", [S, E], F32, isOutput=True)

    Exp = mybir.ActivationFunctionType.Exp

    with tile.TileContext(nc) as tc:
        with (
            tc.tile_pool(name="res", bufs=1) as res,
            tc.tile_pool(name="io", bufs=2) as io,
            tc.tile_pool(name="stp", bufs=3) as stp,
            tc.tile_pool(name="psA", bufs=2, space="PSUM") as psA,
            tc.tile_pool(name="psS", bufs=2, space="PSUM") as psS,
            tc.tile_pool(name="psV", bufs=2, space="PSUM") as psV,
        ):
            # ---- resident small tensors ----
            bq_sb = res.tile([128, N_E], F32, tag="bq")
            bk_sb = res.tile([128, N_E], F32, tag="bk")
            mb_sb = res.tile([128, N_LT], F32, tag="mb")
            nc.sync.dma_start(out=bq_sb, in_=bq.rearrange("(t p) -> p t", p=128))
            nc.sync.dma_start(out=bk_sb, in_=bk.rearrange("(t p) -> p t", p=128))
            nc.sync.dma_start(out=mb_sb, in_=mbias.rearrange("(t p) -> p t", p=128))
            bv_row = res.tile([1, E], MMDT, tag="bvr")
            nc.sync.dma_start(out=bv_row, in_=bv_r.rearrange("(one f) -> one f", one=1))
            ones_f = res.tile([1, 128], F32, tag="ones_f")
            ones_r = res.tile([1, 128], MMDT, tag="ones_r")
            nc.vector.memset(ones_f, 1.0)
            nc.scalar.copy(ones_r, ones_f)
            onesc_f = res.tile([128, H], F32, tag="onesc")
            nc.vector.memset(onesc_f, 1.0)
            # prewarm the ACT exp table set so the first real EXP is fast
            warm = res.tile([1, 8], F32, tag="warm")
            nc.scalar.activation(warm, ones_f[:, 0:8],
                                 mybir.ActivationFunctionType.Exp)
            # per-head-pair broadcast selector (heads 0-7 on rows 0-7, heads
            # 8-15 on rows 32-39 so both reciprocal halves start on legal
            # partitions; rows 8-31 are zero): bcast matmul SEL_hp^T @ recips
            # replicates head h's 1/denom row onto its 64 OT partitions
            SEL = res.tile([40, H * 128 // 2], MMDT, tag="SEL")
            nc.sync.dma_start(out=SEL, in_=selm[:, :])
            # denominator staging: head h parks at partition 32*(h//4),
            # col slot (h%4)*SC (DVE writes need 32-aligned start partitions)
            dn_st = res.tile([97, 4 * SC], MMDT, tag="dnst")

            # ---- long-lived activations + resident weights ----
            KT = [res.tile([128, L], MMDT, tag=f"KT{j}", name=f"KT{j}")
                  for j in range(N_E)]
            # v_aug: head h cols h*65..h*65+64, col h*65+64 == 1.0
            Vt = [res.tile([128, H * (D + 1)], MMDT, tag=f"V{lt}", name=f"V{lt}")
                  for lt in range(N_LT)]
            Wq_sb = [res.tile([128, E], MMDT, tag=f"wq{e}", name=f"wq{e}")
                     for e in range(N_E)]
            Wo_sb = [res.tile([128, E], MMDT, tag=f"wo{e}", name=f"wo{e}")
                     for e in range(N_E)]

            # chunk-0 query + weights prefetch, spread over four engine DMA
            # queues so no single queue gates the prologue
            qin0 = [io.tile([128, SC], MMDT, tag=f"qin{e}", name=f"qin{e}_0")
                    for e in range(N_E)]
            for e in range(N_E):
                nc.gpsimd.dma_start(out=qin0[e], in_=queryT[e * 128:(e + 1) * 128, 0:SC])
            for e in range(4, N_E):
                nc.gpsimd.dma_start(out=Wq_sb[e], in_=Wq[e * 128:(e + 1) * 128, :])
            for e in range(N_E):
                nc.gpsimd.dma_start(out=Wo_sb[e], in_=Wo[e * 128:(e + 1) * 128, :])
            bo_bc = res.tile([128, E], F32, tag="bo")
            bo_ap = bo[:]
            nc.gpsimd.dma_start(
                out=bo_bc,
                in_=bass.AP(tensor=bo_ap.tensor, offset=bo_ap.offset,
                            ap=[[0, 128], [1, E]]),
            )

            # ================= main loop =================
            fillers = deque()

            def pump(n):
                while n > 0 and fillers:
                    try:
                        next(fillers[0])
                        n -= 1
                    except StopIteration:
                        fillers.popleft()

            def drain():
                while fillers:
                    try:
                        next(fillers[0])
                    except StopIteration:
                        fillers.popleft()

            def emit_qt(c, qin, QTt, e_order=None):
                # QT = Wq^T @ queryT + bq, one yield per completed j-chain
                order = list(e_order) if e_order else list(range(N_E))
                for j in range(N_E):
                    ps = psA.tile([128, SC], F32, tag="acc", name=f"qacc{c}_{j}")
                    for i, e in enumerate(order):
                        nc.tensor.matmul(
                            ps, lhsT=Wq_sb[e][:, j * 128:(j + 1) * 128], rhs=qin[e],
                            start=(i == 0), stop=(i == N_E - 1),
                        )
                    nc.vector.tensor_scalar_add(QTt[j], ps, bq_sb[:, j:j + 1])
                    yield

            def emit_outproj(c, OTt):
                # out = OT^T @ Wo + bo, one yield per completed chain
                for st in range(N_LT):
                    sts = slice(st * 128, (st + 1) * 128)
                    for half in range(2):
                        ps = psA.tile([128, SC], F32, tag="acc",
                                      name=f"oacc{c}_{st}_{half}")
                        for j in range(N_E):
                            nc.tensor.matmul(
                                ps, lhsT=OTt[j][:, sts],
                                rhs=Wo_sb[j][:, half * SC:(half + 1) * SC],
                                start=(j == 0), stop=(j == N_E - 1),
                            )
                        ob = stp.tile([128, SC], F32, tag="ob",
                                      name=f"ob{c}_{st}_{half}")
                        nc.vector.tensor_add(
                            ob, ps, bo_bc[:, half * SC:(half + 1) * SC])
                        nc.sync.dma_start(
                            out=out[c * SC + st * 128:c * SC + (st + 1) * 128,
                                    half * SC:(half + 1) * SC],
                            in_=ob,
                        )
                        yield

            def emit_norm(c, OTrw, OTt, rca, hps):
                # per head pair: replicate 1/denom onto the 64 partitions of
                # each head (K=40 selector matmul), then scale the raw PV
                # output: OT = OTraw * bcast   (SBUF bf16 * PSUM f32)
                for hp in hps:
                    psB = psA.tile([128, SC], F32, tag="acc",
                                   name=f"bcp{c}_{hp}")
                    nc.tensor.matmul(
                        psB, lhsT=SEL[:, hp * 128:(hp + 1) * 128], rhs=rca,
                        start=True, stop=True,
                    )
                    nc.vector.tensor_mul(OTt[hp], OTrw[hp], psB)
                    yield

            def emit_scores(c, hp, QTc, atts, pump_lts=(1, 2, 3)):
                att = [io.tile([128, 2 * SC], MMDT, tag=f"att{lt}",
                               name=f"att{lt}_{c}_{hp}", bufs=4)
                       for lt in range(N_LT)]
                for lt in range(N_LT):
                    lts = slice(lt * 128, (lt + 1) * 128)
                    scp = psS.tile([128, 2 * SC], F32, tag="sc",
                                   name=f"sc{c}_{hp}_{lt}")
                    for u in range(2):
                        rows = slice(u * 64, (u + 1) * 64)
                        nc.tensor.matmul(
                            scp[:, u * SC:(u + 1) * SC],
                            lhsT=KT[hp][rows, lts], rhs=QTc[hp][rows, :],
                            start=True, stop=True,
                        )
                    nc.scalar.activation(
                        att[lt], scp, Exp,
                        bias=mb_sb[:, lt:lt + 1], scale=SCALE,
                    )
                    if lt in pump_lts:
                        pump(1)
                atts[hp] = att

            def emit_pv(c, hp, atts, OTraw):
                att = atts.pop(hp)
                for u in range(2):
                    h = 2 * hp + u
                    pv = psV.tile([D + 1, SC], F32, tag="pv", name=f"pv{h}_{c}")
                    for lt in range(N_LT):
                        nc.tensor.matmul(
                            pv, lhsT=Vt[lt][:, h * (D + 1):(h + 1) * (D + 1)],
                            rhs=att[lt][:, u * SC:(u + 1) * SC],
                            start=(lt == 0), stop=(lt == N_LT - 1),
                        )
                    nc.scalar.copy(
                        OTraw[hp][u * 64:(u + 1) * 64, :], pv[0:D, :])
                    nc.vector.tensor_copy(
                        dn_st[32 * (h // 4):32 * (h // 4) + 1,
                              (h % 4) * SC:(h % 4 + 1) * SC],
                        pv[D:D + 1, :])

            # ========== prologue: KT, V, QT(0), early chunk-0 scores ======
            QT_t = {0: [io.tile([128, SC], MMDT, tag=f"QT{j}", name=f"QT{j}_0")
                        for j in range(N_E)]}
            atts0 = {}
            with tc.tile_pool(name="pro", bufs=1) as pro:
                g_in = [pro.tile([128, L], MMDT, tag=f"gin{t}", name=f"gin{t}")
                        for t in range(N_TE)]
                Wk_sb = [pro.tile([128, E], MMDT, tag=f"wk{t}", name=f"wk{t}")
                         for t in range(N_TE)]
                Wv_sb = [pro.tile([128, E], MMDT, tag=f"wv{t}", name=f"wv{t}")
                         for t in range(N_TE)]
                for t in range(N_TE):
                    nc.sync.dma_start(out=g_in[t], in_=guideT[t * 128:(t + 1) * 128, :])
                for t in range(N_TE):
                    nc.scalar.dma_start(out=Wk_sb[t], in_=Wk[t * 128:(t + 1) * 128, :])
                for e in range(4):
                    nc.sync.dma_start(out=Wq_sb[e], in_=Wq[e * 128:(e + 1) * 128, :])
                for t in range(3):
                    nc.sync.dma_start(out=Wv_sb[t], in_=Wv[t * 128:(t + 1) * 128, :])
                for t in range(3, N_TE):
                    nc.scalar.dma_start(out=Wv_sb[t], in_=Wv[t * 128:(t + 1) * 128, :])

                def kt_chain(j):
                    # KT = Wk^T @ guideT + bk
                    ps = psA.tile([128, SC], F32, tag="acc", name=f"kacc{j}")
                    for t in range(N_TE):
                        nc.tensor.matmul(
                            ps, lhsT=Wk_sb[t][:, j * 128:(j + 1) * 128], rhs=g_in[t],
                            start=(t == 0), stop=(t == N_TE - 1),
                        )
                    nc.vector.tensor_scalar_add(KT[j], ps, bk_sb[:, j:j + 1])

                # KT chains first (their DMAs land earliest), then interleave
                # QT(0) chains and the first three chunk-0 score/EXP blocks
                qt0_gen = emit_qt(0, qin0, QT_t[0])
                kt_chain(0)
                kt_chain(1)
                kt_chain(2)
                kt_chain(3)
                next(qt0_gen)
                emit_scores(0, 0, QT_t[0], atts0)
                kt_chain(4); next(qt0_gen)
                emit_scores(0, 1, QT_t[0], atts0)
                kt_chain(5); next(qt0_gen)
                emit_scores(0, 2, QT_t[0], atts0)
                kt_chain(6); next(qt0_gen)
                kt_chain(7); next(qt0_gen)
                for _ in qt0_gen:
                    pass

                # V = guideT^T @ Wv + bv, into v_aug layout
                for lt in range(N_LT):
                    va = Vt[lt].rearrange("p (h c) -> p h c", c=D + 1)
                    for half in range(2):
                        ps = psA.tile([128, SC], F32, tag="acc", name=f"vacc{lt}_{half}")
                        for t in range(N_TE):
                            nc.tensor.matmul(
                                ps, lhsT=g_in[t][:, lt * 128:(lt + 1) * 128],
                                rhs=Wv_sb[t][:, half * SC:(half + 1) * SC],
                                start=(t == 0), stop=False,
                            )
                        nc.tensor.matmul(
                            ps, lhsT=ones_r,
                            rhs=bv_row[:, half * SC:(half + 1) * SC],
                            start=False, stop=True,
                        )
                        nc.vector.tensor_copy(
                            va[:, half * 8:(half + 1) * 8, 0:D],
                            ps.rearrange("p (h c) -> p h c", c=D),
                        )
                    nc.vector.tensor_copy(
                        va[:, :, D:D + 1],
                        onesc_f.rearrange("p (h c) -> p h c", c=1),
                    )

            def make_finalize(c, OTraw, OTc):
                # Finalization is split: heads 0-7 right after PV(c,3) and
                # heads 8-15 after PV(c,7), each a gather DMA + batched
                # reciprocal, so the selector matmuls never wait on a recip.
                state = {}

                def finA():
                    rca = stp.tile([40, SC], MMDT, tag="rca", name=f"rca{c}")
                    dn_c = stp.tile([40, SC], MMDT, tag="dna", name=f"dn_{c}")
                    nc.vector.memset(rca, 0.0)
                    nc.sync.dma_start(out=dn_c[0:4, :], in_=dn_st[0:1, :])
                    nc.sync.dma_start(out=dn_c[4:8, :], in_=dn_st[32:33, :])
                    with nc.allow_low_precision(reason="bf16 recip of denom"):
                        nc.vector.reciprocal(rca[0:8, :], dn_c[0:8, :])
                    state["rca"], state["dn"] = rca, dn_c
                    fillers.append(emit_norm(c, OTraw, OTc, rca, range(0, 4)))

                def finB():
                    rca, dn_c = state["rca"], state["dn"]
                    nc.sync.dma_start(out=dn_c[32:36, :], in_=dn_st[64:65, :])
                    nc.sync.dma_start(out=dn_c[36:40, :], in_=dn_st[96:97, :])
                    with nc.allow_low_precision(reason="bf16 recip of denom"):
                        nc.vector.reciprocal(rca[32:40, :], dn_c[32:40, :])
                    fillers.append(emit_norm(c, OTraw, OTc, rca, range(4, HP)))
                    fillers.append(emit_outproj(c, OTc))
                return finA, finB

            # The last two PV chains of each chunk (and that chunk's
            # finalize) carry over into the next chunk's first score slots so
            # the score/EXP stream never pauses at a chunk boundary.
            carry = []
            for c in range(N_SC):
                if c + 1 < N_SC:
                    qin = [io.tile([128, SC], MMDT, tag=f"qin{e}",
                                   name=f"qin{e}_{c + 1}") for e in range(N_E)]
                    for e in range(N_E):
                        (nc.sync if e % 2 == 0 else nc.gpsimd).dma_start(
                            out=qin[e],
                            in_=queryT[e * 128:(e + 1) * 128,
                                       (c + 1) * SC:(c + 2) * SC])
                    QT_t[c + 1] = [io.tile([128, SC], MMDT, tag=f"QT{j}",
                                           name=f"QT{j}_{c + 1}")
                                   for j in range(N_E)]
                    fillers.append(emit_qt(c + 1, qin, QT_t[c + 1]))
                OTraw = [io.tile([128, SC], MMDT, tag=f"OTr{j}",
                                 name=f"OTr{j}_{c}") for j in range(N_E)]
                OTc = [io.tile([128, SC], MMDT, tag=f"OT{j}", name=f"OT{j}_{c}")
                       for j in range(N_E)]
                QTc = QT_t.pop(c)

                atts = atts0 if c == 0 else {}
                n_own = HP if c == N_SC - 1 else HP - 2
                finA, finB = make_finalize(c, OTraw, OTc)
                fin_map = {3: finA, HP - 1: finB}
                pv_queue = carry + [(c, hp, atts, OTraw, fin_map.get(hp))
                                    for hp in range(n_own)]
                carry = [(c, hp, atts, OTraw, fin_map.get(hp))
                         for hp in range(n_own, HP)]

                for hp in range(3 if c == 0 else 0, HP):
                    if c == 0:
                        # chunk 0's only filler is the 8 QT(1) chains: spread
                        # them 2,2,2,1,1 over its five score slots so the
                        # back half does not starve and go ACT-bound
                        plts = (1, 3) if hp <= 5 else (3,)
                    elif c == N_SC - 1:
                        plts = (3,)
                    else:
                        plts = (1, 2, 3)
                    emit_scores(c, hp, QTc, atts, pump_lts=plts)
                    if pv_queue:
                        pc, php, patts, pOTraw, pfin = pv_queue.pop(0)
                        emit_pv(pc, php, patts, pOTraw)
                        if pfin is not None:
                            pfin()
                while pv_queue:
                    pc, php, patts, pOTraw, pfin = pv_queue.pop(0)
                    emit_pv(pc, php, patts, pOTraw)
                    if pfin is not None:
                        pfin()
            drain()

    if not nc.is_finalized():
        nc.finalize()
    return nc


def kernel(query, guide_vector, attention_mask, Wt, bt, Wq, bq, Wkv, bkv, Wo, bo):
    global _CACHED_NC
    query = np.asarray(query, dtype=np.float32)
    guide_vector = np.asarray(guide_vector, dtype=np.float32)
    attention_mask = np.asarray(attention_mask)
    Wt = np.asarray(Wt, dtype=np.float32)
    bt = np.asarray(bt, dtype=np.float32)
    bq = np.asarray(bq, dtype=np.float32)
    bkv = np.asarray(bkv, dtype=np.float32)
    bo = np.asarray(bo, dtype=np.float32)
    Wkv = np.asarray(Wkv, dtype=np.float32)

    # fold the text projection into the kv projection (host-side, fp32)
    Wf = Wt @ Wkv                       # [TE, 2E]
    bf = bt @ Wkv + bkv                 # [2E]
    Wk_m = np.ascontiguousarray(Wf[:, :E]).astype(NPDT)
    Wv_m = np.ascontiguousarray(Wf[:, E:]).astype(NPDT)
    Wq_m = np.asarray(Wq, dtype=np.float32).astype(NPDT)
    Wo_m = np.asarray(Wo, dtype=np.float32).astype(NPDT)
    bk_m = np.ascontiguousarray(bf[:E])
    bv_m = bf[E:].astype(NPDT)

    if _CACHED_NC is None:
        _CACHED_NC = build_nc()
    nc = _CACHED_NC

    selm = np.zeros((40, H * 128 // 2), dtype=NPDT)
    for h in range(H):
        row = h if h < 8 else 24 + h
        col = (h // 2) * 128 + (h % 2) * 64
        selm[row, col:col + 64] = 1.0

    mb = np.where(attention_mask == 0, np.float32(-1e9), np.float32(0.0))
    in_maps = []
    for b in range(B):
        in_maps.append({
            "queryT": np.ascontiguousarray(query[b].T).astype(NPDT),
            "guideT": np.ascontiguousarray(guide_vector[b].T).astype(NPDT),
            "Wq": Wq_m, "Wk": Wk_m, "Wv": Wv_m, "Wo": Wo_m,
            "bq": bq, "bk": bk_m, "bv_r": bv_m,
            "bo": bo, "mbias": mb[b].astype(np.float32), "selm": selm,
        })
    res = run_bass_kernel_spmd(nc, in_maps, list(range(B)), trace=TRACE)
    if TRACE:
        kernel.last_exec_time_ns = res.exec_time_ns
        kernel.last_results = res
    return np.stack([res.results[b]["out"] for b in range(B)])
